# revision 1
# baseline (speedup 1.0000x reference)
"""Trainium2 Bass kernel for the BMP loss (nn_BMPLoss_24670292148307).

Data-parallel over 8 NeuronCores: each core computes partial sums of every
loss term over its 64 samples; the host combines the 8 partial vectors with
the loss normalization (the global-mean "psum" step).

Per-core device computation:
  - 2D keypoint loss partial  : sum conf*|1000*(pxy/pz) - (g2xy-256)|   (host /512)
  - 3D keypoint loss partial  : sum conf*|pelvis-aligned diff|
  - vertex L1 partial: only mask=1 samples are shipped (packed/balanced on
    host, bf16), streamed [128 x 5814] in 2 chunks; DVE sub + ACT Abs+accum
  - pose / betas squared-diff partials (masked)
  - PA-MPJPE partial: closed-form batched 3x3 Procrustes (trig eigenvalues of
    K^T K via polynomial-seeded Newton on 4x^3-3x=r, smallest eigenvalue
    stabilized as det(K)^2/(l1*l2), Lagrange matrix function for
    V diag(+-1/s) V^T, R = W K^T), vectorized across samples on partitions
  - n_valid partial
The host combines 8x[1,8] partials with the loss normalization constants.
"""
import numpy as np
from contextlib import ExitStack

import concourse.bass as bass
import concourse.bacc as bacc
import concourse.tile as tile
import concourse.mybir as mybir
from concourse.bass_utils import run_bass_kernel_spmd

f32 = mybir.dt.float32
bf16 = mybir.dt.bfloat16
i32 = mybir.dt.int32
AF = mybir.ActivationFunctionType
OP = mybir.AluOpType
AX = mybir.AxisListType

B_PER_CORE = 64
N_CORES = 8
J = 24
VERT_F = 20670          # floats per sample (6890*3)
PACK_CAP = 36           # vertex slots per core (only mask=1 samples shipped;
                        # 264 masked / 8 cores = 33, +margin)
F_PACK = 5814           # ceil(PACK_CAP*VERT_F/128)
N_CHUNK = 2
CHUNK = F_PACK // N_CHUNK  # 2907
EPS = 1e-8

# cos(acos(r)/3) polynomial init (deg 9, chebfit), x3(r) = second polynomial
P1C = [0.8649274597522203, 0.17578197434414333, -0.002087134697444787,
       -0.1271791091353304, -0.3070988770461487, 0.6789215326112841,
       0.5727490378285598, -1.068537975408937, -0.3683220235409602,
       0.5818562170395759]
P3C = [-0.8649274597522203, 0.17578197434414353, 0.002087134697442622,
       -0.1271791091353331, 0.3070988770461617, 0.6789215326112932,
       -0.5727490378285826, -1.068537975408948, 0.3683220235409723,
       0.58185621703958]

TINY = 1e-30


def _consts_array() -> np.ndarray:
    """[64, 32]: cols 0..19 Horner coeff pairs (degree 9 -> 0), cols 20..28 eye(3)."""
    c = np.zeros((B_PER_CORE, 32), np.float32)
    for t in range(10):  # t-th pair is coefficient of degree 9-t
        c[:, 2 * t] = np.float32(P1C[9 - t])
        c[:, 2 * t + 1] = np.float32(P3C[9 - t])
    eye = np.eye(3, dtype=np.float32).reshape(9)
    c[:, 20:29] = eye
    return c


def _emit_det3(nc, pool, M, name):
    """det of batched 3x3 in M [64,9] (row-major cols 3r+c). Returns det [64,1]."""
    V = nc.vector
    P = B_PER_CORE
    Q = pool.tile([P, 9], f32, name=f"q_{name}")
    V.tensor_mul(
        Q[:, :].rearrange("p (a b) -> p a b", a=3),
        M[:, 3:6].unsqueeze(2).broadcast_to([P, 3, 3]),
        M[:, 6:9].unsqueeze(1).broadcast_to([P, 3, 3]),
    )
    D = pool.tile([P, 9], f32, name=f"dq_{name}")
    V.tensor_sub(
        D[:, :].rearrange("p (a b) -> p a b", a=3),
        Q[:, :].rearrange("p (a b) -> p a b", a=3),
        Q[:, :].rearrange("p (b a) -> p a b", b=3),
    )
    u1 = pool.tile([P, 2], f32, name=f"u1_{name}")
    V.tensor_mul(u1[:, :], M[:, 0:2], D[:, 5:7])
    u2 = pool.tile([P, 1], f32, name=f"u2_{name}")
    V.tensor_mul(u2[:, :], M[:, 2:3], D[:, 1:2])
    u1r = pool.tile([P, 1], f32, name=f"u1r_{name}")
    V.tensor_reduce(u1r[:, :], u1[:, :], axis=AX.X, op=OP.add)
    det = pool.tile([P, 1], f32, name=f"det_{name}")
    V.tensor_add(det[:, :], u1r[:, :], u2[:, :])
    return det


def _emit_sqrt(nc, pool, x, n, name, accum_out=None):
    """y = sqrt(x) on ACT (HW-probed table accuracy ~7e-6 rel, sufficient).

    If accum_out is given, the same op writes the per-partition sum(y)."""
    P = B_PER_CORE
    y0 = pool.tile([P, n], f32, name=f"sq0_{name}")
    nc.scalar.activation(y0[:, :], x[:, :], AF.Sqrt, accum_out=accum_out)
    return y0


def build_program(stage: int = 99):
    nc = bacc.Bacc("TRN2", target_bir_lowering=False, debug=False,
                   num_devices=N_CORES)
    P = B_PER_CORE

    # all small fp32 inputs ride in one [64, 727] block, shipped as two DMAs:
    # cols 0:200 (cst|pj|g3 — the procrustes chain's inputs) land first, the
    # rest (cam|g2|rp|rg|pb|gs) second.
    # cols: cst 0:32 | pj 32:104 | g3 104:200 | cam 200:203 | g2 203:275 |
    #       rp 275:491 | rg 491:707 | pb 707:717 | gs 717:727
    blk_d = nc.dram_tensor("blk", [P, 727], f32, kind="ExternalInput")
    hs_d = nc.dram_tensor("hs", [P, 1], i32, kind="ExternalInput")
    va_d = nc.dram_tensor("va", [128, F_PACK], bf16, kind="ExternalInput")
    vb_d = nc.dram_tensor("vb", [128, F_PACK], bf16, kind="ExternalInput")
    out_d = nc.dram_tensor("out", [1, 8], f32, kind="ExternalOutput")

    with tile.TileContext(nc) as tc, ExitStack() as ctx:
        V = nc.vector
        G = nc.gpsimd
        sg_pool = ctx.enter_context(tc.tile_pool(name="singles", bufs=1))
        vpool = ctx.enter_context(tc.tile_pool(name="vpool", bufs=3))
        dpool = ctx.enter_context(tc.tile_pool(name="dpool", bufs=2))
        pp = ctx.enter_context(tc.tile_pool(name="proc", bufs=1))

        def S(shape, name, dtype=f32):
            return sg_pool.tile(list(shape), dtype, name=name)

        comp = S([128, 8], "comp")
        nc.gpsimd.memset(comp[:, :], 0.0)
        vacc = S([128, N_CHUNK], "vacc")

        # First ACT op is a Sqrt so the table loader picks the sqrt set once;
        # Abs/Copy are filler functions present in every set.
        warm = S([1, 1], "warm")
        G.memset(warm[:, :], 1.0)
        warm2 = S([1, 1], "warm2")
        nc.scalar.activation(warm2[:, :], warm[:, :], AF.Sqrt)

        # ---------------- small inputs ----------------
        blk_t = S([P, 727], "blk_t")
        nc.sync.dma_start(blk_t[:, 0:200], blk_d[:, 0:200])
        nc.sync.dma_start(blk_t[:, 200:727], blk_d[:, 200:727])
        hs_t = S([P, 1], "hs_t", i32)
        nc.sync.dma_start(hs_t[:, :], hs_d[:, :])
        cst_t = blk_t[:, 0:32]
        pj_t = blk_t[:, 32:104]
        g3_t = blk_t[:, 104:200]
        cam_t = blk_t[:, 200:203]
        g2_t = blk_t[:, 203:275]
        rp_t = blk_t[:, 275:491]
        rg_t = blk_t[:, 491:707]
        pb_t = blk_t[:, 707:717]
        gs_t = blk_t[:, 717:727]
        eye9 = cst_t[:, 20:29]
        # ------- vertex stream (mask=1 samples packed on host), emitted
        # ------- last so the serial procrustes chain gets DVE/DMA priority
        for c in range(N_CHUNK):
            sl = slice(c * CHUNK, (c + 1) * CHUNK)
            va_t = vpool.tile([128, CHUNK], bf16, name="va_t", tag="va")
            nc.sync.dma_start(va_t[:, :], va_d[:, sl])
            vb_t = vpool.tile([128, CHUNK], bf16, name="vb_t", tag="vb")
            nc.sync.dma_start(vb_t[:, :], vb_d[:, sl])
            d_t = dpool.tile([128, CHUNK], bf16, name="d_t", tag="d")
            V.tensor_sub(d_t[:, :], va_t[:, :], vb_t[:, :])
            s_t = dpool.tile([128, CHUNK], bf16, name="s_t", tag="s")
            nc.scalar.activation(s_t[:, :], d_t[:, :], AF.Abs,
                                 accum_out=vacc[:, c:c + 1])
        # vertex per-partition total (packed data is all weight-1)
        V.tensor_reduce(comp[:, 2:3], vacc[:, :], axis=AX.X, op=OP.add)

        # ---------------- mask ----------------
        hsf = S([P, 1], "hsf")
        G.tensor_copy(hsf[:, :], hs_t[:, :])
        mask_f = S([P, 1], "mask_f")
        G.tensor_single_scalar(mask_f[:, :], hsf[:, :], 0.5, OP.is_gt)
        G.tensor_copy(comp[0:P, 6:7], mask_f[:, :])

        pj_r = pj_t[:, :].rearrange("p (n i) -> p n i", i=3)
        g2_r = g2_t[:, :].rearrange("p (n i) -> p n i", i=3)
        g3_r = g3_t[:, :].rearrange("p (n i) -> p n i", i=4)

        # ---------------- kp2d ----------------
        # ================ Procrustes ================
        if stage >= 4:
            musum1 = pp.tile([P, 3], f32, name="musum1")
            V.tensor_reduce(musum1[:, :], pj_t[:, :].rearrange(
                "p (n i) -> p i n", i=3), axis=AX.X, op=OP.add)
            musum2 = pp.tile([P, 3], f32, name="musum2")
            V.tensor_reduce(
                musum2[:, :],
                g3_t[:, :].rearrange("p (n i) -> p i n", i=4)[:, 0:3, :],
                axis=AX.X, op=OP.add)

            X1n = pp.tile([P, 72], f32, name="X1n")
            V.scalar_tensor_tensor(
                X1n[:, :].rearrange("p (n i) -> p n i", i=3),
                musum1[:, :].unsqueeze(1).broadcast_to([P, J, 3]), 1.0 / J,
                pj_r, OP.mult, OP.subtract)
            X2n = pp.tile([P, 72], f32, name="X2n")
            V.scalar_tensor_tensor(
                X2n[:, :].rearrange("p (n i) -> p n i", i=3),
                musum2[:, :].unsqueeze(1).broadcast_to([P, J, 3]), 1.0 / J,
                g3_r[:, :, 0:3], OP.mult, OP.subtract)

            var1 = pp.tile([P, 1], f32, name="var1")
            scrv = pp.tile([P, 72], f32, name="scrv")
            V.tensor_mul(scrv[:, :], X1n[:, :], X1n[:, :])
            V.tensor_reduce(var1[:, :], scrv[:, :], axis=AX.X, op=OP.add)

            kprod = pp.tile([P, 216], f32, name="kprod")
            V.tensor_mul(
                kprod[:, :].rearrange("p (i j n) -> p i j n", i=3, j=3),
                X1n[:, :].rearrange("p (n i) -> p i n", i=3)
                    .unsqueeze(2).broadcast_to([P, 3, 3, J]),
                X2n[:, :].rearrange("p (n j) -> p j n", j=3)
                    .unsqueeze(1).broadcast_to([P, 3, 3, J]))
            # K = X1^T X2; the reference's +1e-8 on O(10) fp32 entries is
            # below fp32 resolution, so it is omitted
            K9 = pp.tile([P, 9], f32, name="K9")
            V.tensor_reduce(K9[:, :], kprod[:, :].rearrange(
                "p (i j n) -> p i j n", i=3, j=3), axis=AX.X, op=OP.add)

            aprod = pp.tile([P, 27], f32, name="aprod")
            V.tensor_mul(
                aprod[:, :].rearrange("p (i j k) -> p i j k", i=3, j=3),
                K9[:, :].rearrange("p (k i) -> p i k", k=3)
                    .unsqueeze(2).broadcast_to([P, 3, 3, 3]),
                K9[:, :].rearrange("p (k j) -> p j k", k=3)
                    .unsqueeze(1).broadcast_to([P, 3, 3, 3]))
            A9 = pp.tile([P, 9], f32, name="A9")
            V.tensor_reduce(A9[:, :], aprod[:, :].rearrange(
                "p (i j k) -> p i j k", i=3, j=3), axis=AX.X, op=OP.add)

            detK = _emit_det3(nc, pp, K9, "k")
            if stage == 4:
                V.tensor_copy(comp[0:P, 7:8], detK[:, :])

        if stage >= 5:
            qsum = pp.tile([P, 1], f32, name="qsum")
            V.tensor_reduce(qsum[:, :], A9[:, 0:9:4], axis=AX.X, op=OP.add)
            qthird = pp.tile([P, 1], f32, name="qthird")
            V.tensor_single_scalar(qthird[:, :], qsum[:, :], 1.0 / 3.0,
                                   OP.mult)
            aqn = pp.tile([P, 9], f32, name="aqn")  # qI - A (negated Aq)
            V.scalar_tensor_tensor(aqn[:, :], eye9, qthird[:, :], A9[:, :],
                                   OP.mult, OP.subtract)
            p2r = pp.tile([P, 1], f32, name="p2r")
            scrp2 = pp.tile([P, 9], f32, name="scrp2")
            V.tensor_mul(scrp2[:, :], aqn[:, :], aqn[:, :])
            V.tensor_reduce(p2r[:, :], scrp2[:, :], axis=AX.X, op=OP.add)
            p2g = pp.tile([P, 1], f32, name="p2g")
            V.tensor_scalar(p2g[:, :], p2r[:, :], 1.0 / 6.0, TINY, OP.mult,
                            OP.max)
            pp_ = _emit_sqrt(nc, pp, p2g, 1, "p")
            pinv = pp.tile([P, 1], f32, name="pinv")
            V.reciprocal(pinv[:, :], pp_[:, :])
            bmn = pp.tile([P, 9], f32, name="bmn")
            V.tensor_scalar_mul(bmn[:, :], aqn[:, :], pinv[:, :])
            detBn = _emit_det3(nc, pp, bmn, "b")
            r0 = pp.tile([P, 1], f32, name="r0")
            V.tensor_scalar(r0[:, :], detBn[:, :], -0.5, 1.0, OP.mult, OP.min)
            rr = pp.tile([P, 1], f32, name="rr")
            V.tensor_single_scalar(rr[:, :], r0[:, :], -1.0, OP.max)

            # Horner seed for both roots of 4x^3 - 3x = r
            x = pp.tile([P, 2], f32, name="xroots")
            V.scalar_tensor_tensor(x[:, :], cst_t[:, 0:2], rr[:, :],
                                   cst_t[:, 2:4], OP.mult, OP.add)
            for t in range(2, 10):
                V.scalar_tensor_tensor(x[:, :], x[:, :], rr[:, :],
                                       cst_t[:, 2 * t:2 * t + 2],
                                       OP.mult, OP.add)
            # Newton as x' = (8x^3 + r)/(12x^2 - 3)
            x2t = pp.tile([P, 2], f32, name="x2t")
            x3t = pp.tile([P, 2], f32, name="x3t")
            num = pp.tile([P, 2], f32, name="num")
            dh = pp.tile([P, 2], f32, name="dh")
            dinv = pp.tile([P, 2], f32, name="dinv")
            for _ in range(2):
                V.tensor_mul(x2t[:, :], x[:, :], x[:, :])
                V.tensor_mul(x3t[:, :], x2t[:, :], x[:, :])
                V.scalar_tensor_tensor(num[:, :], x3t[:, :], 8.0,
                                       rr[:, :].broadcast_to([P, 2]),
                                       OP.mult, OP.add)
                V.tensor_scalar(dh[:, :], x2t[:, :], 12.0, -3.0, OP.mult,
                                OP.add)
                V.tensor_single_scalar(dh[:, :], dh[:, :], 1e-4, OP.max)
                V.reciprocal(dinv[:, :], dh[:, :])
                V.tensor_mul(x[:, :], num[:, :], dinv[:, :])

            twop = pp.tile([P, 1], f32, name="twop")
            V.tensor_single_scalar(twop[:, :], pp_[:, :], 2.0, OP.mult)
            ls3 = pp.tile([P, 3], f32, name="ls3")
            # L1 -> col0, trig L3 -> col2 (later replaced by detK^2/(L1*L2))
            V.scalar_tensor_tensor(ls3[:, 0:3:2], x[:, :], twop[:, :],
                                   qthird[:, :].broadcast_to([P, 2]),
                                   OP.mult, OP.add)
            l13s = pp.tile([P, 1], f32, name="l13s")
            V.tensor_reduce(l13s[:, :], ls3[:, 0:3:2], axis=AX.X, op=OP.add)
            V.tensor_sub(ls3[:, 1:2], qsum[:, :], l13s[:, :])
            t12 = pp.tile([P, 1], f32, name="t12")
            V.tensor_mul(t12[:, :], ls3[:, 0:1], ls3[:, 1:2])
            t12g = pp.tile([P, 1], f32, name="t12g")
            V.tensor_single_scalar(t12g[:, :], t12[:, :], TINY, OP.max)
            rt12 = pp.tile([P, 1], f32, name="rt12")
            V.reciprocal(rt12[:, :], t12g[:, :])
            dk2 = pp.tile([P, 1], f32, name="dk2")
            V.tensor_mul(dk2[:, :], detK[:, :], detK[:, :])
            V.tensor_mul(ls3[:, 2:3], dk2[:, :], rt12[:, :])
            V.tensor_single_scalar(ls3[:, :], ls3[:, :], TINY, OP.max)

            s3t = _emit_sqrt(nc, pp, ls3, 3, "s")
            sinv = pp.tile([P, 3], f32, name="sinv")
            V.reciprocal(sinv[:, :], s3t[:, :])
            sg0 = pp.tile([P, 1], f32, name="sg0")
            V.tensor_single_scalar(sg0[:, :], detK[:, :], 0.0, OP.is_ge)
            sgn = pp.tile([P, 1], f32, name="sgn")
            V.tensor_scalar(sgn[:, :], sg0[:, :], 2.0, -1.0, OP.mult, OP.add)
            if stage == 5:
                V.tensor_copy(comp[0:P, 7:8], s3t[:, 2:3])

        if stage >= 6:
            lsI = pp.tile([P, 27], f32, name="lsI")
            V.tensor_mul(lsI[:, :].rearrange("p (m x) -> p m x", m=3),
                         ls3[:, :].unsqueeze(2).broadcast_to([P, 3, 9]),
                         eye9.unsqueeze(1).broadcast_to([P, 3, 9]))
            mstack = pp.tile([P, 27], f32, name="mstack")
            V.tensor_sub(mstack[:, :].rearrange("p (m x) -> p m x", m=3),
                         A9[:, :].unsqueeze(1).broadcast_to([P, 3, 9]),
                         lsI[:, :].rearrange("p (m x) -> p m x", m=3))

            mr = mstack[:, :].rearrange("p (m a k) -> p m a k", m=3, a=3)
            pms = []
            for nm, (ba, bb) in (("pm1", (1, 2)), ("pm2", (0, 2)),
                                 ("pm3", (0, 1))):
                prod = pp.tile([P, 27], f32, name=f"prod_{nm}")
                V.tensor_mul(
                    prod[:, :].rearrange("p (a b k) -> p a b k", a=3, b=3),
                    mr[:, ba].unsqueeze(2).broadcast_to([P, 3, 3, 3]),
                    mr[:, bb].transpose([0, 2, 1]).unsqueeze(1)
                        .broadcast_to([P, 3, 3, 3]))
                pm = pp.tile([P, 9], f32, name=nm)
                V.tensor_reduce(pm[:, :], prod[:, :].rearrange(
                    "p (a b k) -> p a b k", a=3, b=3), axis=AX.X, op=OP.add)
                pms.append(pm)

            g12 = pp.tile([P, 1], f32, name="g12")
            V.tensor_sub(g12[:, :], ls3[:, 0:1], ls3[:, 1:2])
            g13 = pp.tile([P, 1], f32, name="g13")
            V.tensor_sub(g13[:, :], ls3[:, 0:1], ls3[:, 2:3])
            g23 = pp.tile([P, 1], f32, name="g23")
            V.tensor_sub(g23[:, :], ls3[:, 1:2], ls3[:, 2:3])
            dvec = pp.tile([P, 3], f32, name="dvec")
            V.tensor_mul(dvec[:, 0:1], g12[:, :], g13[:, :])
            V.tensor_mul(dvec[:, 1:2], g12[:, :], g23[:, :])
            V.tensor_mul(dvec[:, 2:3], g13[:, :], g23[:, :])
            dvi = pp.tile([P, 3], f32, name="dvi")
            V.reciprocal(dvi[:, :], dvec[:, :])
            cv = pp.tile([P, 3], f32, name="cv")
            V.tensor_mul(cv[:, :], sinv[:, :], dvi[:, :])
            V.tensor_mul(cv[:, 2:3], cv[:, 2:3], sgn[:, :])
            V.tensor_single_scalar(cv[:, 1:2], cv[:, 1:2], -1.0, OP.mult)

            W = pp.tile([P, 9], f32, name="W")
            V.tensor_scalar_mul(W[:, :], pms[0][:, :], cv[:, 0:1])
            V.scalar_tensor_tensor(W[:, :], pms[1][:, :], cv[:, 1:2], W[:, :],
                                   OP.mult, OP.add)
            V.scalar_tensor_tensor(W[:, :], pms[2][:, :], cv[:, 2:3], W[:, :],
                                   OP.mult, OP.add)

            rprod = pp.tile([P, 27], f32, name="rprod")
            V.tensor_mul(
                rprod[:, :].rearrange("p (a b k) -> p a b k", a=3, b=3),
                W[:, :].rearrange("p (a k) -> p a k", a=3)
                    .unsqueeze(2).broadcast_to([P, 3, 3, 3]),
                K9[:, :].rearrange("p (b k) -> p b k", b=3)
                    .unsqueeze(1).broadcast_to([P, 3, 3, 3]))
            R9 = pp.tile([P, 9], f32, name="R9")
            V.tensor_reduce(R9[:, :], rprod[:, :].rearrange(
                "p (a b k) -> p a b k", a=3, b=3), axis=AX.X, op=OP.add)
            if stage == 6:
                V.tensor_copy(comp[0:P, 7:8], R9[:, 0:1])

        if stage >= 7:
            ssum = pp.tile([P, 1], f32, name="ssum")
            V.tensor_add(ssum[:, :], s3t[:, 0:1], s3t[:, 1:2])
            s3g = pp.tile([P, 1], f32, name="s3g")
            V.tensor_mul(s3g[:, :], s3t[:, 2:3], sgn[:, :])
            V.tensor_add(ssum[:, :], ssum[:, :], s3g[:, :])
            v1i = pp.tile([P, 1], f32, name="v1i")
            V.reciprocal(v1i[:, :], var1[:, :])
            scl = pp.tile([P, 1], f32, name="scl")
            V.tensor_mul(scl[:, :], ssum[:, :], v1i[:, :])

            rxprod = pp.tile([P, 216], f32, name="rxprod")
            V.tensor_mul(
                rxprod[:, :].rearrange("p (i n j) -> p i n j", i=3, n=J),
                X1n[:, :].rearrange("p (n j) -> p n j", j=3)
                    .unsqueeze(1).broadcast_to([P, 3, J, 3]),
                R9[:, :].rearrange("p (i j) -> p i j", i=3)
                    .unsqueeze(2).broadcast_to([P, 3, J, 3]))
            rx1 = pp.tile([P, 72], f32, name="rx1")
            V.tensor_reduce(rx1[:, :].rearrange("p (n i) -> p i n", i=3),
                            rxprod[:, :].rearrange("p (i n j) -> p i n j",
                                                   i=3, n=J),
                            axis=AX.X, op=OP.add)
            Y = pp.tile([P, 72], f32, name="Y")
            V.scalar_tensor_tensor(Y[:, :], rx1[:, :], scl[:, :], X2n[:, :],
                                   OP.mult, OP.subtract)
            Y2 = pp.tile([P, 72], f32, name="Y2")
            V.tensor_mul(Y2[:, :], Y[:, :], Y[:, :])
            d2 = pp.tile([P, J], f32, name="d2")
            V.tensor_reduce(d2[:, :],
                            Y2[:, :].rearrange("p (n i) -> p n i", i=3),
                            axis=AX.X, op=OP.add)
            _emit_sqrt(nc, pp, d2, J, "d", accum_out=comp[0:P, 5:6])

        if stage >= 2:
            t1 = S([P, 1], "t1")
            V.tensor_scalar(t1[:, :], cam_t[:, 0:1], 512.0, EPS, OP.mult,
                            OP.add)
            rt1 = S([P, 1], "rt1")
            V.reciprocal(rt1[:, :], t1[:, :])
            depth = S([P, 1], "depth")
            V.tensor_single_scalar(depth[:, :], rt1[:, :], 2000.0, OP.mult)
            pxy = S([P, 48], "pxy")
            V.tensor_add(pxy[:, :].rearrange("p (n i) -> p n i", i=2),
                         pj_r[:, :, 0:2],
                         cam_t[:, 1:3].unsqueeze(1).broadcast_to([P, J, 2]))
            pz = S([P, J], "pz")
            V.tensor_single_scalar(pz[:, :], pj_r[:, :, 2].squeeze(),
                                   depth[:, :], OP.add)
            rz = S([P, J], "rz")
            V.reciprocal(rz[:, :], pz[:, :])
            aa = S([P, 48], "aa")
            V.tensor_mul(aa[:, :].rearrange("p (n i) -> p n i", i=2),
                         pxy[:, :].rearrange("p (n i) -> p n i", i=2),
                         rz[:, :].unsqueeze(2).broadcast_to([P, J, 2]))
            g2s = S([P, 48], "g2s")
            V.tensor_single_scalar(g2s[:, :].rearrange("p (n i) -> p n i", i=2),
                                   g2_r[:, :, 0:2], 256.0, OP.subtract)
            dkp = S([P, 48], "dkp")
            V.scalar_tensor_tensor(dkp[:, :], aa[:, :], 1000.0, g2s[:, :],
                                   OP.mult, OP.subtract)
            u2d = S([P, 48], "u2d")
            V.tensor_mul(u2d[:, :].rearrange("p (n i) -> p n i", i=2),
                         dkp[:, :].rearrange("p (n i) -> p n i", i=2),
                         g2_r[:, :, 2:3].broadcast_to([P, J, 2]))
            scr2d = S([P, 48], "scr2d")
            nc.scalar.activation(scr2d[:, :], u2d[:, :], AF.Abs,
                                 accum_out=comp[0:P, 0:1])

            # ---------------- kp3d ----------------
            pd = S([P, 72], "pd")
            V.tensor_sub(pd[:, :].rearrange("p (n i) -> p n i", i=3),
                         pj_r, g3_r[:, :, 0:3])
            pel = S([P, 3], "pel")
            V.tensor_add(pel[:, :], pd[:, 6:9], pd[:, 9:12])
            d3n = S([P, 72], "d3n")
            V.scalar_tensor_tensor(
                d3n[:, :].rearrange("p (n i) -> p n i", i=3),
                pel[:, :].unsqueeze(1).broadcast_to([P, J, 3]), 0.5,
                pd[:, :].rearrange("p (n i) -> p n i", i=3),
                OP.mult, OP.subtract)
            u3d = S([P, 72], "u3d")
            V.tensor_mul(u3d[:, :].rearrange("p (n i) -> p n i", i=3),
                         d3n[:, :].rearrange("p (n i) -> p n i", i=3),
                         g3_r[:, :, 3:4].broadcast_to([P, J, 3]))
            scr3d = S([P, 72], "scr3d")
            nc.scalar.activation(scr3d[:, :], u3d[:, :], AF.Abs,
                                 accum_out=comp[0:P, 1:2])

        # ---------------- pose / betas ----------------
        if stage >= 3:
            dp = S([P, 216], "dp")
            V.tensor_sub(dp[:, :], rp_t[:, :], rg_t[:, :])
            scrp = S([P, 216], "scrp")
            pose_per = S([P, 1], "pose_per")
            nc.scalar.activation(scrp[:, :], dp[:, :], AF.Square,
                                 accum_out=pose_per[:, :])
            V.tensor_mul(comp[0:P, 3:4], pose_per[:, :], mask_f[:, :])

            db = S([P, 10], "db")
            V.tensor_sub(db[:, :], pb_t[:, :], gs_t[:, :])
            scrb = S([P, 10], "scrb")
            betas_per = S([P, 1], "betas_per")
            nc.scalar.activation(scrb[:, :], db[:, :], AF.Square,
                                 accum_out=betas_per[:, :])
            V.tensor_mul(comp[0:P, 4:5], betas_per[:, :], mask_f[:, :])


        # ---------------- final cross-partition reduce ----------------
        ones_t = S([128, 1], "ones_t")
        G.memset(ones_t[:, :], 1.0)
        psum_pool = ctx.enter_context(
            tc.tile_pool(name="psum", bufs=1, space="PSUM"))
        ps = psum_pool.tile([1, 8], f32, name="ps")
        nc.tensor.matmul(ps[:, :], ones_t[:, :], comp[:, :], start=True,
                         stop=True)
        out_s = S([1, 8], "out_s")
        V.tensor_copy(out_s[:, :], ps[:, :])
        nc.sync.dma_start(out_d[:, :], out_s[:, :])

    nc.compile()
    return nc


_PROGRAM = None


def _get_program():
    global _PROGRAM
    if _PROGRAM is None:
        _PROGRAM = build_program()
    return _PROGRAM


def make_in_maps(inputs: dict) -> list:
    pj = np.ascontiguousarray(np.asarray(inputs["pred_joints"], np.float32))
    cam = np.ascontiguousarray(np.asarray(inputs["pred_camera"], np.float32))
    g2 = np.ascontiguousarray(np.asarray(inputs["gt_keypoints_2d"], np.float32))
    g3 = np.ascontiguousarray(np.asarray(inputs["gt_keypoints_3d"], np.float32))
    rp = np.ascontiguousarray(np.asarray(inputs["pred_rotmat"], np.float32))
    rg = np.ascontiguousarray(np.asarray(inputs["gt_rotmat"], np.float32))
    pb = np.ascontiguousarray(np.asarray(inputs["pred_betas"], np.float32))
    gs = np.ascontiguousarray(np.asarray(inputs["gt_shape"], np.float32))
    hs = np.ascontiguousarray(np.asarray(inputs["has_smpl"], np.int32))
    va = np.asarray(inputs["pred_vertices"], np.float32).reshape(512, VERT_F)
    vb = np.asarray(inputs["gt_vertices"], np.float32).reshape(512, VERT_F)
    cst = _consts_array()

    # pack mask=1 samples' vertices, balanced round-robin across cores
    idx = np.nonzero(hs > 0)[0]
    assert idx.size <= N_CORES * PACK_CAP, (
        f"n_valid={idx.size} exceeds vertex pack capacity "
        f"{N_CORES * PACK_CAP}; increase PACK_CAP")

    import ml_dtypes

    def packed(src, sel):
        buf = np.zeros(128 * F_PACK, ml_dtypes.bfloat16)
        if sel.size:
            flat = src[sel].reshape(-1)
            buf[:flat.size] = flat.astype(ml_dtypes.bfloat16)
        return buf.reshape(128, F_PACK)

    in_maps = []
    for c in range(N_CORES):
        sl = slice(B_PER_CORE * c, B_PER_CORE * (c + 1))
        sel = idx[c::N_CORES]
        blk = np.concatenate([
            cst,
            pj[sl].reshape(B_PER_CORE, 72),
            g3[sl].reshape(B_PER_CORE, 96),
            cam[sl],
            g2[sl].reshape(B_PER_CORE, 72),
            rp[sl].reshape(B_PER_CORE, 216),
            rg[sl].reshape(B_PER_CORE, 216),
            pb[sl],
            gs[sl],
        ], axis=1)
        assert blk.shape == (B_PER_CORE, 727)
        in_maps.append({
            "blk": np.ascontiguousarray(blk, np.float32),
            "hs": hs[sl].reshape(B_PER_CORE, 1),
            "va": packed(va, sel),
            "vb": packed(vb, sel),
        })
    return in_maps


def combine_partials(parts: np.ndarray) -> np.float32:
    s = parts.astype(np.float64).sum(0)
    kp2d, kp3d, vert, pose, betas, pa, nv = s[:7]
    B = 512.0
    total = (4.0 * kp2d / (512.0 * B * J * 2)
             + 4.0 * kp3d / (B * J * 3)
             + vert / (nv * 6890 * 3 + EPS)
             + pose / (nv * 24 * 9 + EPS)
             + 0.01 * betas / (nv * 10 + EPS)
             + pa / (B * J))
    return np.float32(total)


def kernel(**inputs) -> np.ndarray:
    nc = _get_program()
    in_maps = make_in_maps(inputs)
    res = run_bass_kernel_spmd(nc, in_maps, core_ids=list(range(N_CORES)))
    parts = np.stack([res.results[c]["out"][0] for c in range(N_CORES)])
    return np.asarray(combine_partials(parts))



# revision 22
# speedup vs baseline: 1.0266x; 1.0266x over previous
"""Trainium2 Bass kernel for the BMP loss (nn_BMPLoss_24670292148307). V2.

Data-parallel over 8 NeuronCores; host combines per-core partial sums.

V2 redesign vs the 34us baseline (DVE small-op chain dominated):
  - vertex L1: masked samples shipped fp8(e4m3), gt NEGATED on host; all four
    vertex DMAs are gpsimd-issued on one queue as [va0, +vbn0, va1, +vbn1]
    where the vbn transfers use accum_op=add, so pred-gt materializes in SBUF
    with zero compute-engine cost and in-queue ordering (no semaphore hops).
    ACT does Abs+accumulate per chunk.
  - all small inputs ride one bf16 block; pj/g3 pre-transposed to (xyz,joint)
    on host so both Procrustes centroids come from one reduce and K comes
    from one mul+reduce.
  - Procrustes: r = det((A-qI)/p)/2 via det(A-qI) = detA - q^3 + 3*q*p^2
    (detA = detK^2); Horner + 1 Newton for the outer cosine roots (r clamped
    to +-0.99995 keeps the Newton denominator 12x^2-3 positive: no clamp op);
    lam_mid by trace identity; lam3 = detA/(lam1*lam2); eigenvector
    reconstruction in monomial form W = a2*A^2 + a1*A + a0*I (alphas from
    [64,3] column math) instead of the Lagrange matrix products.
  - kp2d prep on gpsimd; var1/pose/betas/kp losses accumulate on ACT
    (Square/Abs with accum_out) in its idle windows.
  - output: comp[128,8] DMAed out directly; host does the final scaling.
"""
import numpy as np
from contextlib import ExitStack

import concourse.bass as bass
import concourse.bacc as bacc
import concourse.tile as tile
import concourse.mybir as mybir
from concourse.bass_utils import run_bass_kernel_spmd

f32 = mybir.dt.float32
bf16 = mybir.dt.bfloat16
fp8 = mybir.dt.float8e4
AF = mybir.ActivationFunctionType
OP = mybir.AluOpType
AX = mybir.AxisListType

B_PER_CORE = 64
N_CORES = 8
J = 24
VERT_F = 20670           # floats per sample (6890*3)
PACK_CAP = 34            # vertex slots per core (33 used at n_valid=264)
N_CHUNK = 2
CH = 2746
F_PACK = N_CHUNK * CH    # 5492 >= ceil(34*20670/128) = 5491
EPS = 1e-8
TINY = 1e-30
RCLAMP = 0.99995

# blk (bf16) column map
PG6 = slice(0, 144)      # (c,n): rows 0-2 pj xyz, 3-5 gt3 xyz, joint-minor
CONF3 = slice(144, 168)
CAM = slice(168, 171)
G2 = slice(171, 219)     # (c,n), pre-shifted by -256
CONF2 = slice(219, 243)
RP = slice(243, 459)
RG = slice(459, 675)
PB = slice(675, 685)
GS = slice(685, 695)
BLK_COLS = 695

# cst (f32) column map
HC = slice(0, 20)        # Horner pairs (P1C[9-t], P3C[9-t]) per degree step
EYE9 = slice(20, 29)
EYE3 = slice(29, 38)     # eye/3 (for qI - A with q = qsum/3)
MASKC = slice(38, 39)
CST_COLS = 40

P1C = [0.8649274597522203, 0.17578197434414333, -0.002087134697444787,
       -0.1271791091353304, -0.3070988770461487, 0.6789215326112841,
       0.5727490378285598, -1.068537975408937, -0.3683220235409602,
       0.5818562170395759]
P3C = [-0.8649274597522203, 0.17578197434414353, 0.002087134697442622,
       -0.1271791091353331, 0.3070988770461617, 0.6789215326112932,
       -0.5727490378285826, -1.068537975408948, 0.3683220235409723,
       0.58185621703958]


def _cst_array() -> np.ndarray:
    c = np.zeros((B_PER_CORE, CST_COLS), np.float32)
    for t in range(10):
        c[:, 2 * t] = np.float32(P1C[9 - t])
        c[:, 2 * t + 1] = np.float32(P3C[9 - t])
    eye = np.eye(3, dtype=np.float32).reshape(9)
    c[:, EYE9] = eye
    c[:, EYE3] = eye / 3.0
    return c


def build_program():
    nc = bacc.Bacc("TRN2", target_bir_lowering=False, debug=False,
                   num_devices=N_CORES)
    P = B_PER_CORE

    cst_d = nc.dram_tensor("cst", [P, CST_COLS], f32, kind="ExternalInput")
    blk_d = nc.dram_tensor("blk", [P, BLK_COLS], bf16, kind="ExternalInput")
    va_d = nc.dram_tensor("va", [128, F_PACK], fp8, kind="ExternalInput")
    vbn_d = nc.dram_tensor("vbn", [128, F_PACK], fp8, kind="ExternalInput")
    out_d = nc.dram_tensor("out", [128, 8], f32, kind="ExternalOutput")

    with tile.TileContext(nc) as tc, ExitStack() as ctx:
        V = nc.vector
        A = nc.scalar
        G = nc.gpsimd
        SP = nc.sync
        sg = ctx.enter_context(tc.tile_pool(name="singles", bufs=1))
        vp = ctx.enter_context(tc.tile_pool(name="vp", bufs=2))

        def S(shape, name, dtype=f32):
            return sg.tile(list(shape), dtype, name=name)

        comp = S([128, 8], "comp")
        G.memset(comp[:, :], 0.0)
        vacc = S([128, N_CHUNK], "vacc")

        # first ACT op is a Sqrt so the table loader picks the sqrt set once
        warm = S([1, 1], "warm")
        G.memset(warm[:, :], 1.0)
        warm2 = S([1, 1], "warm2")
        A.activation(warm2[:, :], warm[:, :], AF.Sqrt)

        # ---------------- input DMAs ----------------------------------------
        # small inputs via SP (HWDGE)
        blk_t = S([P, BLK_COLS], "blk_t", bf16)
        SP.dma_start(blk_t[:, :], blk_d[:, :])
        cst_t = S([P, CST_COLS], "cst_t")
        SP.dma_start(cst_t[:, :], cst_d[:, :])
        t1 = S([P, 1], "t1")
        # vertex stream: gpsimd SWDGE, one queue, accum pairs
        import os
        VERT_MODE = os.environ.get("VERT_MODE", "accum")
        d_ts = []
        vb_ts = []
        for c in range(N_CHUNK):
            sl = slice(c * CH, (c + 1) * CH)
            d_t = vp.tile([128, CH], fp8, name=f"d{c}", tag="d")
            if VERT_MODE == "accum":
                G.dma_start(d_t[:, :], va_d[:, sl])
                # vbn holds -gt: DMA-accumulate computes pred - gt in flight
                G.dma_start(d_t[:, :], vbn_d[:, sl], accum_op=OP.add)
            else:
                va_t = vp.tile([128, CH], fp8, name=f"va{c}", tag="va")
                SP.dma_start(va_t[:, :], va_d[:, sl])
                vb_t = vp.tile([128, CH], fp8, name=f"vb{c}", tag="vb")
                SP.dma_start(vb_t[:, :], vbn_d[:, sl])
                vb_ts.append((va_t, vb_t, d_t))
            d_ts.append(d_t)

        pg6 = blk_t[:, PG6]
        eye9 = cst_t[:, EYE9]
        eye3 = cst_t[:, EYE3]
        maskf = cst_t[:, MASKC]

        # ================ DVE chain ================
        musum = S([P, 6], "musum")
        V.tensor_reduce(musum[:, :], pg6.rearrange("p (c n) -> p c n", n=J),
                        axis=AX.X, op=OP.add)
        Xn = S([P, 144], "Xn")     # (musum/24 - pg6): negated centered coords
        V.scalar_tensor_tensor(
            Xn[:, :].rearrange("p (c n) -> p c n", n=J),
            musum[:, :].unsqueeze(2).broadcast_to([P, 6, J]), 1.0 / J,
            pg6.rearrange("p (c n) -> p c n", n=J), OP.mult, OP.subtract)
        X1n = Xn[:, 0:72]
        X2n = Xn[:, 72:144]
        var1 = S([P, 1], "var1")
        vscr = S([P, 72], "vscr")
        A.activation(vscr[:, :], X1n, AF.Square, accum_out=var1[:, :])
        V.tensor_scalar(t1[:, :], blk_t[:, CAM][:, 0:1], 512.0, EPS,
                        OP.mult, OP.add)
        rt1 = S([P, 1], "rt1")
        V.reciprocal(rt1[:, :], t1[:, :])

        # K = X1 X2^T
        kq = S([P, 216], "kq")
        V.tensor_mul(
            kq[:, :].rearrange("p (i j n) -> p i j n", i=3, j=3),
            X1n.rearrange("p (i n) -> p i n", i=3)
                .unsqueeze(2).broadcast_to([P, 3, 3, J]),
            X2n.rearrange("p (j n) -> p j n", j=3)
                .unsqueeze(1).broadcast_to([P, 3, 3, J]))
        K9 = S([P, 9], "K9")
        V.tensor_reduce(K9[:, :], kq[:, :].rearrange(
            "p (i j n) -> p i j n", i=3, j=3), axis=AX.X, op=OP.add)

        # det(K) on DVE (feeds detA for r, and the sign)
        dQ = S([P, 9], "dQ")
        V.tensor_mul(
            dQ[:, :].rearrange("p (a b) -> p a b", a=3),
            K9[:, 3:6].unsqueeze(2).broadcast_to([P, 3, 3]),
            K9[:, 6:9].unsqueeze(1).broadcast_to([P, 3, 3]))
        dD = S([P, 9], "dD")
        V.tensor_sub(dD[:, :].rearrange("p (a b) -> p a b", a=3),
                     dQ[:, :].rearrange("p (a b) -> p a b", a=3),
                     dQ[:, :].rearrange("p (b a) -> p a b", b=3))
        du1 = S([P, 2], "du1")
        V.tensor_mul(du1[:, :], K9[:, 0:2], dD[:, 5:7])
        du2 = S([P, 1], "du2")
        V.tensor_mul(du2[:, :], K9[:, 2:3], dD[:, 1:2])
        du1r = S([P, 1], "du1r")
        V.tensor_reduce(du1r[:, :], du1[:, :], axis=AX.X, op=OP.add)
        detK = S([P, 1], "detK")
        V.tensor_add(detK[:, :], du1r[:, :], du2[:, :])
        detA = S([P, 1], "detA")
        V.tensor_mul(detA[:, :], detK[:, :], detK[:, :])
        sg0 = S([P, 1], "sg0")
        V.tensor_single_scalar(sg0[:, :], detK[:, :], 0.0, OP.is_ge)
        sgn = S([P, 1], "sgn")
        V.tensor_scalar(sgn[:, :], sg0[:, :], 2.0, -1.0, OP.mult, OP.add)

        # A = K^T K
        aq = S([P, 27], "aq")
        V.tensor_mul(
            aq[:, :].rearrange("p (i j k) -> p i j k", i=3, j=3),
            K9[:, :].rearrange("p (k i) -> p i k", k=3)
                .unsqueeze(2).broadcast_to([P, 3, 3, 3]),
            K9[:, :].rearrange("p (k j) -> p j k", k=3)
                .unsqueeze(1).broadcast_to([P, 3, 3, 3]))
        A9 = S([P, 9], "A9")
        V.tensor_reduce(A9[:, :], aq[:, :].rearrange(
            "p (i j k) -> p i j k", i=3, j=3), axis=AX.X, op=OP.add)
        qsum = S([P, 1], "qsum")
        V.tensor_reduce(qsum[:, :], A9[:, 0:9:4], axis=AX.X, op=OP.add)
        q3rd = S([P, 1], "q3rd")
        V.tensor_single_scalar(q3rd[:, :], qsum[:, :], 1.0 / 3.0, OP.mult)
        q2 = S([P, 1], "q2")
        V.tensor_mul(q2[:, :], q3rd[:, :], q3rd[:, :])
        q3 = S([P, 1], "q3")
        V.tensor_mul(q3[:, :], q2[:, :], q3rd[:, :])
        nqsum = S([P, 1], "nqsum")
        V.tensor_single_scalar(nqsum[:, :], qsum[:, :], -1.0, OP.mult)
        aqn = S([P, 9], "aqn")
        V.scalar_tensor_tensor(aqn[:, :], eye3, qsum[:, :], A9[:, :],
                               OP.mult, OP.subtract)
        pscr = S([P, 9], "pscr")
        V.tensor_mul(pscr[:, :], aqn[:, :], aqn[:, :])
        p2r = S([P, 1], "p2r")
        V.tensor_reduce(p2r[:, :], pscr[:, :], axis=AX.X, op=OP.add)
        p2g = S([P, 1], "p2g")
        V.tensor_scalar(p2g[:, :], p2r[:, :], 1.0 / 6.0, TINY,
                        OP.mult, OP.max)
        pp = S([P, 1], "pp")
        A.activation(pp[:, :], p2g[:, :], AF.Sqrt)
        # 2p on Pool right after the sqrt
        tp = S([P, 1], "tp")
        G.tensor_single_scalar(tp[:, :], pp[:, :], 2.0, OP.mult)

        # z = detA - q^3 + 3 q p^2 and the kp3d block fill the sqrt wait
        zu = S([P, 1], "zu")
        V.tensor_mul(zu[:, :], q3rd[:, :], p2g[:, :])
        zv = S([P, 1], "zv")
        V.scalar_tensor_tensor(zv[:, :], zu[:, :], 3.0, q3[:, :],
                               OP.mult, OP.subtract)
        zz = S([P, 1], "zz")
        V.tensor_add(zz[:, :], detA[:, :], zv[:, :])

        # ---------------- kp3d (DVE, fills the ACT-sqrt wait) ---------------
        pd = S([P, 72], "pd", bf16)
        V.tensor_sub(pd[:, :], blk_t[:, 0:72], blk_t[:, 72:144])
        pdr = pd[:, :].rearrange("p (c n) -> p c n", n=J)
        pel = S([P, 3], "pel", bf16)
        V.tensor_add(pel[:, :], pdr[:, :, 2].squeeze(), pdr[:, :, 3].squeeze())
        d3n = S([P, 72], "d3n", bf16)
        V.scalar_tensor_tensor(
            d3n[:, :].rearrange("p (c n) -> p c n", n=J),
            pel[:, :].unsqueeze(2).broadcast_to([P, 3, J]), 0.5,
            pdr, OP.mult, OP.subtract)
        u3d = S([P, 72], "u3d", bf16)
        V.tensor_mul(u3d[:, :].rearrange("p (c n) -> p c n", n=J),
                     d3n[:, :].rearrange("p (c n) -> p c n", n=J),
                     blk_t[:, CONF3].unsqueeze(1).broadcast_to([P, 3, J]))
        kscr3 = S([P, 72], "kscr3")
        A.activation(kscr3[:, :], u3d[:, :], AF.Abs,
                     accum_out=comp[0:P, 1:2])

        # pose/betas subs (DVE) + Square-accumulate (ACT idle window)
        dp = S([P, 216], "dp", bf16)
        V.tensor_sub(dp[:, :], blk_t[:, RP], blk_t[:, RG])
        pscr2 = S([P, 216], "pscr2", bf16)
        pose_per = S([P, 1], "pose_per")
        A.activation(pscr2[:, :], dp[:, :], AF.Square,
                     accum_out=pose_per[:, :])
        db = S([P, 10], "db", bf16)
        V.tensor_sub(db[:, :], blk_t[:, PB], blk_t[:, GS])
        bscr = S([P, 10], "bscr", bf16)
        betas_per = S([P, 1], "betas_per")
        A.activation(bscr[:, :], db[:, :], AF.Square,
                     accum_out=betas_per[:, :])

        pinv = S([P, 1], "pinv")
        V.reciprocal(pinv[:, :], pp[:, :])
        pv2 = S([P, 1], "pv2")
        V.tensor_mul(pv2[:, :], pinv[:, :], pinv[:, :])
        pv3 = S([P, 1], "pv3")
        V.tensor_mul(pv3[:, :], pv2[:, :], pinv[:, :])
        r0 = S([P, 1], "r0")
        V.tensor_mul(r0[:, :], zz[:, :], pv3[:, :])
        r1 = S([P, 1], "r1")
        V.tensor_scalar(r1[:, :], r0[:, :], 0.5, RCLAMP, OP.mult, OP.min)
        rr = S([P, 1], "rr")
        V.tensor_single_scalar(rr[:, :], r1[:, :], -RCLAMP, OP.max)

        # Horner seed for outer roots [c1, c3]
        x = S([P, 2], "xroots")
        V.scalar_tensor_tensor(x[:, :], cst_t[:, 0:2], rr[:, :],
                               cst_t[:, 2:4], OP.mult, OP.add)
        for t in range(2, 10):
            V.scalar_tensor_tensor(x[:, :], x[:, :], rr[:, :],
                                   cst_t[:, 2 * t:2 * t + 2],
                                   OP.mult, OP.add)
        # one Newton step: x' = (8x^3 + r)/(12x^2 - 3); den > 0 given RCLAMP
        x2t = S([P, 2], "x2t")
        V.tensor_mul(x2t[:, :], x[:, :], x[:, :])
        x3t = S([P, 2], "x3t")
        V.tensor_mul(x3t[:, :], x2t[:, :], x[:, :])
        nm = S([P, 2], "nm")
        V.scalar_tensor_tensor(nm[:, :], x3t[:, :], 8.0,
                               rr[:, :].broadcast_to([P, 2]),
                               OP.mult, OP.add)
        dh = S([P, 2], "dh")
        V.tensor_scalar(dh[:, :], x2t[:, :], 12.0, -3.0, OP.mult, OP.add)
        dinv = S([P, 2], "dinv")
        V.reciprocal(dinv[:, :], dh[:, :])
        V.tensor_mul(x[:, :], nm[:, :], dinv[:, :])

        # eigenvalues: lam = [l1, lmid, l3=detA/(l1*lmid)], clamped >= TINY
        lamt = S([P, 3], "lamt")
        V.scalar_tensor_tensor(lamt[:, 0:3:2], x[:, :], tp[:, :],
                               q3rd[:, :].broadcast_to([P, 2]),
                               OP.mult, OP.add)
        t13 = S([P, 1], "t13")
        V.tensor_add(t13[:, :], lamt[:, 0:1], lamt[:, 2:3])
        V.tensor_sub(lamt[:, 1:2], qsum[:, :], t13[:, :])
        t12 = S([P, 1], "t12")
        V.tensor_mul(t12[:, :], lamt[:, 0:1], lamt[:, 1:2])
        t12g = S([P, 1], "t12g")
        V.tensor_single_scalar(t12g[:, :], t12[:, :], TINY, OP.max)
        rt12 = S([P, 1], "rt12")
        V.reciprocal(rt12[:, :], t12g[:, :])
        V.tensor_mul(lamt[:, 2:3], detA[:, :], rt12[:, :])
        lam = S([P, 3], "lam")
        V.tensor_single_scalar(lam[:, :], lamt[:, :], TINY, OP.max)
        s3t = S([P, 3], "s3t")
        A.activation(s3t[:, :], lam[:, :], AF.Sqrt)

        # v1i here (var1 ready long ago; needed only for scl)
        v1i = S([P, 1], "v1i")
        V.reciprocal(v1i[:, :], var1[:, :])
        # kp2d prep (Pool after the SWDGE gens; rzt in the s3-wait filler)
        depth = S([P, 1], "depth")
        G.tensor_single_scalar(depth[:, :], rt1[:, :], 2000.0, OP.mult)
        pxy = S([P, 48], "pxy", bf16)
        G.tensor_add(pxy[:, :].rearrange("p (c n) -> p c n", n=J),
                     blk_t[:, PG6].rearrange("p (c n) -> p c n", n=J)[:, 0:2],
                     blk_t[:, CAM][:, 1:3].unsqueeze(2).broadcast_to([P, 2, J]))
        pzt = S([P, J], "pzt")
        G.tensor_add(pzt[:, :], blk_t[:, 48:72],
                     depth[:, :].broadcast_to([P, J]))
        rzt = S([P, J], "rzt")
        V.reciprocal(rzt[:, :], pzt[:, :])
        aa = S([P, 48], "aa")
        G.tensor_mul(aa[:, :].rearrange("p (c n) -> p c n", n=J),
                     pxy[:, :].rearrange("p (c n) -> p c n", n=J),
                     rzt[:, :].unsqueeze(1).broadcast_to([P, 2, J]))
        # host ships g2' = (g2-256)/1000 and conf2' = conf*1000, so the
        # 1000x projection scale folds into the confidence weight
        dkp = S([P, 48], "dkp")
        G.tensor_sub(dkp[:, :], aa[:, :], blk_t[:, G2])
        u2d = S([P, 48], "u2d")
        G.tensor_mul(u2d[:, :].rearrange("p (c n) -> p c n", n=J),
                     dkp[:, :].rearrange("p (c n) -> p c n", n=J),
                     blk_t[:, CONF2].unsqueeze(1).broadcast_to([P, 2, J]))

        sinv = S([P, 3], "sinv")
        V.reciprocal(sinv[:, :], s3t[:, :])
        gA = S([P, 2], "gA")   # [l1-lmid, lmid-l3]
        V.tensor_sub(gA[:, :], lam[:, 0:2], lam[:, 1:3])
        g02 = S([P, 1], "g02")
        V.tensor_add(g02[:, :], gA[:, 0:1], gA[:, 1:2])
        Dt = S([P, 3], "Dt")   # signed gap products
        V.tensor_mul(Dt[:, 0:1], gA[:, 0:1], g02[:, :])
        V.scalar_tensor_tensor(Dt[:, 1:2], gA[:, 0:1], -1.0, gA[:, 1:2],
                               OP.mult, OP.mult)
        V.tensor_mul(Dt[:, 2:3], g02[:, :], gA[:, 1:2])
        rD = S([P, 3], "rD")
        V.reciprocal(rD[:, :], Dt[:, :])
        mv = S([P, 3], "mv")
        V.tensor_mul(mv[:, :], rD[:, :], sinv[:, :])
        V.tensor_mul(mv[:, 2:3], mv[:, 2:3], sgn[:, :])

        # A^2 and the monomial alphas
        a2q = S([P, 27], "a2q")
        V.tensor_mul(
            a2q[:, :].rearrange("p (i j k) -> p i j k", i=3, j=3),
            A9[:, :].rearrange("p (i k) -> p i k", i=3)
                .unsqueeze(2).broadcast_to([P, 3, 3, 3]),
            A9[:, :].rearrange("p (k j) -> p j k", k=3)
                .unsqueeze(1).broadcast_to([P, 3, 3, 3]))
        A29 = S([P, 9], "A29")
        V.tensor_reduce(A29[:, :], a2q[:, :].rearrange(
            "p (i j k) -> p i j k", i=3, j=3), axis=AX.X, op=OP.add)
        al2 = S([P, 1], "al2")
        V.tensor_reduce(al2[:, :], mv[:, :], axis=AX.X, op=OP.add)
        mscr = S([P, 3], "mscr")
        V.tensor_mul(mscr[:, :], mv[:, :], lam[:, :])
        tml = S([P, 1], "tml")
        V.tensor_reduce(tml[:, :], mscr[:, :], axis=AX.X, op=OP.add)
        al1 = S([P, 1], "al1")
        V.scalar_tensor_tensor(al1[:, :], al2[:, :], nqsum[:, :], tml[:, :],
                               OP.mult, OP.add)
        linv = S([P, 3], "linv")
        V.tensor_mul(linv[:, :], sinv[:, :], sinv[:, :])
        mscr2 = S([P, 3], "mscr2")
        V.tensor_mul(mscr2[:, :], mv[:, :], linv[:, :])
        tm0 = S([P, 1], "tm0")
        V.tensor_reduce(tm0[:, :], mscr2[:, :], axis=AX.X, op=OP.add)
        al0 = S([P, 1], "al0")
        V.tensor_mul(al0[:, :], tm0[:, :], detA[:, :])

        aI = S([P, 9], "aI")
        V.tensor_scalar_mul(aI[:, :], eye9, al0[:, :])
        W1 = S([P, 9], "W1")
        V.scalar_tensor_tensor(W1[:, :], A29[:, :], al2[:, :], aI[:, :],
                               OP.mult, OP.add)
        W9 = S([P, 9], "W9")
        V.scalar_tensor_tensor(W9[:, :], A9[:, :], al1[:, :], W1[:, :],
                               OP.mult, OP.add)

        # scale chain (DVE; small): scl = (s1+s2+sgn*s3)/var1 * pinv^2 / 3
        s2s = S([P, 1], "s2s")
        V.tensor_mul(s2s[:, :], s3t[:, 2:3], sgn[:, :])
        s01 = S([P, 1], "s01")
        V.tensor_add(s01[:, :], s3t[:, 0:1], s3t[:, 1:2])
        ssum = S([P, 1], "ssum")
        V.tensor_add(ssum[:, :], s01[:, :], s2s[:, :])
        sw1 = S([P, 1], "sw1")
        V.tensor_mul(sw1[:, :], ssum[:, :], v1i[:, :])
        sw2 = S([P, 1], "sw2")
        V.tensor_mul(sw2[:, :], sw1[:, :], pv2[:, :])
        scl = S([P, 1], "scl")
        V.tensor_single_scalar(scl[:, :], sw2[:, :], 1.0 / 3.0, OP.mult)

        # R = W K^T ; RX1 ; Y ; d2
        rq = S([P, 27], "rq")
        V.tensor_mul(
            rq[:, :].rearrange("p (a b c) -> p a b c", a=3, b=3),
            W9[:, :].rearrange("p (a c) -> p a c", a=3)
                .unsqueeze(2).broadcast_to([P, 3, 3, 3]),
            K9[:, :].rearrange("p (b c) -> p b c", b=3)
                .unsqueeze(1).broadcast_to([P, 3, 3, 3]))
        R9 = S([P, 9], "R9")
        V.tensor_reduce(R9[:, :], rq[:, :].rearrange(
            "p (a b c) -> p a b c", a=3, b=3), axis=AX.X, op=OP.add)
        rxq = S([P, 216], "rxq")
        V.tensor_mul(
            rxq[:, :].rearrange("p (i n j) -> p i n j", i=3, n=J),
            R9[:, :].rearrange("p (i j) -> p i j", i=3)
                .unsqueeze(2).broadcast_to([P, 3, J, 3]),
            X1n.rearrange("p (j n) -> p n j", j=3)
                .unsqueeze(1).broadcast_to([P, 3, J, 3]))
        rx1 = S([P, 72], "rx1")
        V.tensor_reduce(rx1[:, :].rearrange("p (i n) -> p i n", i=3),
                        rxq[:, :].rearrange("p (i n j) -> p i n j",
                                            i=3, n=J),
                        axis=AX.X, op=OP.add)
        Yt = S([P, 72], "Yt")
        V.scalar_tensor_tensor(Yt[:, :], rx1[:, :], scl[:, :], X2n,
                               OP.mult, OP.subtract)
        Y2 = S([P, 72], "Y2")
        V.tensor_mul(Y2[:, :], Yt[:, :], Yt[:, :])
        d2 = S([P, J], "d2")
        V.tensor_reduce(d2[:, :],
                        Y2[:, :].rearrange("p (i n) -> p n i", i=3),
                        axis=AX.X, op=OP.add)

        # ---------------- vertex abs+accumulate (ACT) -----------------------
        for c in range(N_CHUNK):
            if VERT_MODE != "accum":
                va_t, vb_t, d_t = vb_ts[c]
                V.tensor_add(d_t[:, :], va_t[:, :], vb_t[:, :])
            s_t = vp.tile([128, CH], fp8, name=f"s{c}", tag="s")
            A.activation(s_t[:, :], d_ts[c][:, :], AF.Abs,
                         accum_out=vacc[:, c:c + 1])
        # kp2d and pa accumulations close the ACT queue
        kscr = S([P, 48], "kscr")
        A.activation(kscr[:, :], u2d[:, :], AF.Abs,
                     accum_out=comp[0:P, 0:1])
        dscr = S([P, J], "dscr")
        A.activation(dscr[:, :], d2[:, :], AF.Sqrt,
                     accum_out=comp[0:P, 5:6])

        # masked pose/betas into comp (Pool)
        G.tensor_mul(comp[0:P, 3:4], pose_per[:, :], maskf)
        G.tensor_mul(comp[0:P, 4:5], betas_per[:, :], maskf)
        V.tensor_reduce(comp[:, 2:3], vacc[:, :], axis=AX.X, op=OP.add)

        # ---------------- output --------------------------------------------
        SP.dma_start(out_d[:, :], comp[:, :])

    nc.compile()
    return nc


_PROGRAM = None


def _get_program():
    global _PROGRAM
    if _PROGRAM is None:
        _PROGRAM = build_program()
    return _PROGRAM


def make_in_maps(inputs: dict) -> list:
    import ml_dtypes

    pj = np.asarray(inputs["pred_joints"], np.float32)
    cam = np.asarray(inputs["pred_camera"], np.float32)
    g2 = np.asarray(inputs["gt_keypoints_2d"], np.float32)
    g3 = np.asarray(inputs["gt_keypoints_3d"], np.float32)
    rp = np.asarray(inputs["pred_rotmat"], np.float32).reshape(512, 216)
    rg = np.asarray(inputs["gt_rotmat"], np.float32).reshape(512, 216)
    pb = np.asarray(inputs["pred_betas"], np.float32)
    gs = np.asarray(inputs["gt_shape"], np.float32)
    hs = np.asarray(inputs["has_smpl"], np.int32)
    va = np.asarray(inputs["pred_vertices"], np.float32).reshape(512, VERT_F)
    vb = np.asarray(inputs["gt_vertices"], np.float32).reshape(512, VERT_F)
    cst = _cst_array()

    idx = np.nonzero(hs > 0)[0]
    assert idx.size <= N_CORES * PACK_CAP, (
        f"n_valid={idx.size} exceeds vertex pack capacity")

    def packed(src, sel, negate):
        buf = np.zeros(128 * F_PACK, ml_dtypes.float8_e4m3fn)
        if sel.size:
            flat = src[sel].reshape(-1)
            if negate:
                flat = -flat
            buf[:flat.size] = flat.astype(ml_dtypes.float8_e4m3fn)
        return buf.reshape(128, F_PACK)

    in_maps = []
    for c in range(N_CORES):
        sl = slice(B_PER_CORE * c, B_PER_CORE * (c + 1))
        sel = idx[c::N_CORES]
        blk = np.empty((B_PER_CORE, BLK_COLS), np.float32)
        blk[:, 0:72] = pj[sl].transpose(0, 2, 1).reshape(B_PER_CORE, 72)
        blk[:, 72:144] = g3[sl, :, :3].transpose(0, 2, 1).reshape(
            B_PER_CORE, 72)
        blk[:, CONF3] = g3[sl, :, 3]
        blk[:, CAM] = cam[sl]
        blk[:, G2] = ((g2[sl, :, :2] - 256.0) / 1000.0).transpose(
            0, 2, 1).reshape(B_PER_CORE, 48)
        blk[:, CONF2] = g2[sl, :, 2] * 1000.0
        blk[:, RP] = rp[sl]
        blk[:, RG] = rg[sl]
        blk[:, PB] = pb[sl]
        blk[:, GS] = gs[sl]
        cstc = cst.copy()
        cstc[:, MASKC] = (hs[sl] > 0).astype(np.float32)[:, None]
        in_maps.append({
            "cst": np.ascontiguousarray(cstc, np.float32),
            "blk": np.ascontiguousarray(blk.astype(ml_dtypes.bfloat16)),
            "va": packed(va, sel, False),
            "vbn": packed(vb, sel, True),
        })
    return in_maps


def combine_partials(parts: np.ndarray, n_valid: float) -> np.float32:
    # parts: [n_cores, 128, 8]
    s = parts.astype(np.float64).sum((0, 1))
    kp2d, kp3d, vert, pose, betas, pa = s[:6]
    B = 512.0
    total = (4.0 * kp2d / (512.0 * B * J * 2)
             + 4.0 * kp3d / (B * J * 3)
             + vert / (n_valid * VERT_F + EPS)
             + pose / (n_valid * 216 + EPS)
             + 0.01 * betas / (n_valid * 10 + EPS)
             + pa / (B * J))
    return np.float32(total)


def kernel(**inputs) -> np.ndarray:
    nc = _get_program()
    in_maps = make_in_maps(inputs)
    res = run_bass_kernel_spmd(nc, in_maps, core_ids=list(range(N_CORES)))
    parts = np.stack([res.results[c]["out"] for c in range(N_CORES)])
    nv = float((np.asarray(inputs["has_smpl"]) > 0).sum())
    return np.asarray(combine_partials(parts, nv))
